# revision 8
# baseline (speedup 1.0000x reference)
"""Tensor-parallel GQA attention layer for 8 TRN2 NeuronCores.

Sharding: TP over heads. Core c owns q-heads 4c..4c+3 (512 cols of Wq),
kv-head c (128 cols of Wk/Wv), and rows 512c..512c+512 of Wo. Activations
(hidden_states) are replicated; each core emits a partial O-projection
[2048, 4096] which the host sums.

On-device math (per core), all matmuls bf16 with fp32 PSUM accumulate:
  QT[d,s] = Wq_c^T X^T   (4 tiles of [128,2048])
  KT[d,s] = Wk_c^T X^T ; VT[d,s] = Wv_c^T X^T ; V = VT^T via PE transpose
  per (head, s-block of 512):
    S^T[t,s]  = K Q_h^T           (16 t-chunk matmuls, K^T chunks stationary)
    P^T       = exp(S^T * scale)  (ACT, psum->sbuf bf16)
    r[1,s]    = ones^T P^T        (ones stationary, accumulate over t)
    U^T[d,s]  = V^T P^T           (V chunks stationary, accumulate over t)
    A^T       = U^T * bcast(1/r)  (PE K=1 broadcast matmul + DVE multiply)
  out[s,:]   += A^T_chunks^T Wo_c  (partial; host sums cores, adds bo/bv terms)
"""

import sys

if "/opt/trn_rl_repo" not in sys.path:
    sys.path.insert(0, "/opt/trn_rl_repo")

from contextlib import ExitStack

import ml_dtypes
import numpy as np

H = 4096
NH = 32
NKV = 8
HD = 128
S = 2048
NCORES = 8
QH = NH // NCORES          # 4 q heads per core
DQ = QH * HD               # 512
SCALE = float(HD) ** -0.5
P = 128
NSB = S // 512             # 4 s-blocks
NHC = H // P               # 32 hidden chunks
NTT = S // P               # 16 t-chunks
BF16 = ml_dtypes.bfloat16

_CACHE: dict = {}


def _build():
    import concourse.bacc as bacc
    import concourse.mybir as mybir
    import concourse.tile as tile
    from concourse.masks import make_identity

    bf = mybir.dt.bfloat16
    f32 = mybir.dt.float32
    AF = mybir.ActivationFunctionType
    ALU = mybir.AluOpType

    nc = bacc.Bacc("TRN2", target_bir_lowering=False, debug=False)

    xt_d = nc.dram_tensor("xt", [H, S], bf, kind="ExternalInput")
    wq_d = nc.dram_tensor("wq", [H, DQ], bf, kind="ExternalInput")
    wk_d = nc.dram_tensor("wk", [H, HD], bf, kind="ExternalInput")
    wv_d = nc.dram_tensor("wv", [H, HD], bf, kind="ExternalInput")
    wo_d = nc.dram_tensor("wo", [DQ, H], bf, kind="ExternalInput")
    bias_d = nc.dram_tensor("bias", [P, 6], f32, kind="ExternalInput")
    out_d = nc.dram_tensor("out", [S, H], bf, kind="ExternalOutput")

    with tile.TileContext(nc) as tc, ExitStack() as ctx:
        const = ctx.enter_context(tc.tile_pool(name="const", bufs=1))
        ones_col = const.tile([P, 1], bf)
        nc.vector.memset(ones_col[:], 1.0)
        ones_row = const.tile([1, P], bf)
        nc.vector.memset(ones_row[:], 1.0)
        ident = const.tile([P, P], bf)
        make_identity(nc, ident[:])
        zbias = const.tile([P, 1], f32)
        nc.vector.memset(zbias[:], 0.0)
        bias_sb = const.tile([P, 6], f32)
        nc.sync.dma_start(bias_sb[:], bias_d[:])

        keep = ctx.enter_context(tc.tile_pool(name="keep", bufs=1))
        qt = [keep.tile([P, S], bf, tag=f"qt{i}", name=f"qt{i}") for i in range(QH)]
        kt = keep.tile([P, S], bf, tag="kt")
        vt = keep.tile([P, S], bf, tag="vt")
        vv = keep.tile([P, NTT, P], bf, tag="vv")      # V chunks [t,d]
        at = [keep.tile([P, S], bf, tag=f"at{i}", name=f"at{i}") for i in range(QH)]

        # ---- Phase A: projections. X^T fully resident; weights streamed.
        # s-blocks are the inner matmul loop so each stationary weight chunk
        # serves 4 matmuls (one LDWEIGHTS per 4 MMs instead of per MM). ----
        with ExitStack() as ca:
            xp = ca.enter_context(tc.tile_pool(name="xp", bufs=1))
            wp = ca.enter_context(tc.tile_pool(name="wp", bufs=8))
            pa = ca.enter_context(tc.tile_pool(name="pa", bufs=1, space="PSUM"))

            xs = xp.tile([P, NHC, S], bf)
            xt_v = xt_d[:].rearrange("(a p) s -> p a s", p=P)
            for g in range(8):
                gs = slice(g * 4, (g + 1) * 4)
                nc.sync.dma_start(xs[:, gs, :], xt_v[:, gs, :])
            wq_v = wq_d[:].rearrange("(a p) n -> p a n", p=P)
            wk_v = wk_d[:].rearrange("(a p) n -> p a n", p=P)
            wv_v = wv_d[:].rearrange("(a p) n -> p a n", p=P)

            # passes: (dram view, col offset, [(dst, bias col, head off)...])
            passes = [
                (wq_v, 0 * HD, 2 * HD, [(qt[0], 0), (qt[1], 1)]),
                (wq_v, 2 * HD, 2 * HD, [(qt[2], 2), (qt[3], 3)]),
                (wk_v, 0, HD, [(kt, 4)]),
                (wv_v, 0, HD, [(vt, 5)]),
            ]
            for pi, (wv_view, coff, width, outs) in enumerate(passes):
                nheads = len(outs)
                pss = [
                    [
                        pa.tile([P, 512], f32, tag=f"pa{hi * NSB + sb}", name="ps")
                        for sb in range(NSB)
                    ]
                    for hi in range(nheads)
                ]
                for h in range(NHC):
                    wt = wp.tile([P, width], bf, tag=f"w{width}", name="wt")
                    nc.sync.dma_start(wt[:], wv_view[:, h, coff : coff + width])
                    for hi in range(nheads):
                        for sb in range(NSB):
                            nc.tensor.matmul(
                                pss[hi][sb][:],
                                lhsT=wt[:, hi * HD : (hi + 1) * HD],
                                rhs=xs[:, h, sb * 512 : (sb + 1) * 512],
                                start=(h == 0),
                                stop=(h == NHC - 1),
                            )
                for hi, (dst, bcol) in enumerate(outs):
                    for sb in range(NSB):
                        nc.scalar.activation(
                            dst[:, sb * 512 : (sb + 1) * 512],
                            pss[hi][sb][:],
                            AF.Identity,
                            bias=bias_sb[:, bcol : bcol + 1],
                        )

            # V = VT^T via PE transpose (16 chunks); reuse pa slots
            for t in range(NTT):
                pt_ps = pa.tile([P, P], bf, tag=f"pa{t % 4}", name="pt_ps")
                nc.tensor.transpose(pt_ps[:], vt[:, t * P : (t + 1) * P], ident[:])
                nc.vector.tensor_copy(vv[:, t, :], pt_ps[:])

        # Wo staged early so its DMA overlaps phase B compute.
        wop = ctx.enter_context(tc.tile_pool(name="wop", bufs=1))
        wo_sb = wop.tile([P, QH, H], bf)
        nc.sync.dma_start(wo_sb[:], wo_d[:].rearrange("(a p) n -> p a n", p=P))

        # ---- Phase B: attention, software-pipelined over 16 (head, s-block)
        # blocks. Score matmuls of block i interleave with rowsum/AV matmuls
        # of block i-1 so the PE never waits on ACT's exp. Exp runs on
        # 4-chunk groups ([128, 2048]) to amortize ACT fixed overhead. ----
        with ExitStack() as cb:
            pb = cb.enter_context(tc.tile_pool(name="pb", bufs=1))
            ss = cb.enter_context(tc.tile_pool(name="ss", bufs=1, space="PSUM"))
            us = cb.enter_context(tc.tile_pool(name="us", bufs=2, space="PSUM"))
            rs = cb.enter_context(tc.tile_pool(name="rs", bufs=1, space="PSUM"))
            bs = cb.enter_context(tc.tile_pool(name="bs", bufs=1, space="PSUM"))

            blocks = [(hq, sb) for hq in range(QH) for sb in range(NSB)]

            def score_group(pt_i, hq, sb, g):
                sl = slice(sb * 512, (sb + 1) * 512)
                sps = ss.tile([P, 4, 512], f32, tag="ss", name="sps")
                for j in range(4):
                    t = 4 * g + j
                    nc.tensor.matmul(
                        sps[:, j, :],
                        lhsT=kt[:, t * P : (t + 1) * P],
                        rhs=qt[hq][:, sl],
                        start=True,
                        stop=True,
                    )
                nc.scalar.activation(
                    pt_i[:, 4 * g : 4 * g + 4, :],
                    sps[:],
                    AF.Exp,
                    bias=zbias[:],
                    scale=SCALE,
                )

            def consume_t(pt_p, rps, ups, t):
                nc.tensor.matmul(
                    rps[:],
                    lhsT=ones_col[:],
                    rhs=pt_p[:, t, :],
                    start=(t == 0),
                    stop=(t == NTT - 1),
                )
                nc.tensor.matmul(
                    ups[:],
                    lhsT=vv[:, t, :],
                    rhs=pt_p[:, t, :],
                    start=(t == 0),
                    stop=(t == NTT - 1),
                )

            def normalize(rps, ups, hq, sb):
                sl = slice(sb * 512, (sb + 1) * 512)
                recip = pb.tile([1, 512], f32, tag="recip", bufs=2, name="recip")
                nc.vector.reciprocal(recip[:], rps[:])
                recip_bf = pb.tile([1, 512], bf, tag="recipb", bufs=2, name="recipb")
                nc.vector.tensor_copy(recip_bf[:], recip[:])
                bps = bs.tile([P, 512], f32, tag="bs", name="bps")
                nc.tensor.matmul(
                    bps[:], lhsT=ones_row[:], rhs=recip_bf[:], start=True, stop=True
                )
                bcast = pb.tile([P, 512], f32, tag="bcast", bufs=2, name="bcast")
                nc.vector.tensor_copy(bcast[:], bps[:])
                nc.vector.scalar_tensor_tensor(
                    out=at[hq][:, sl],
                    in0=ups[:],
                    scalar=1.0,
                    in1=bcast[:],
                    op0=ALU.mult,
                    op1=ALU.mult,
                )

            prev = None
            for i in range(len(blocks) + 1):
                cur = None
                if i < len(blocks):
                    hq, sb = blocks[i]
                    pt_i = pb.tile([P, NTT, 512], bf, tag="pt", bufs=2, name="pt")
                    cur = (pt_i, hq, sb)
                if prev is not None:
                    pt_p, phq, psb = prev
                    rps = rs.tile([1, 512], f32, tag="rs", name="rps")
                    ups = us.tile([P, 512], f32, tag="us", name="ups")
                    for g in range(4):
                        if cur is not None:
                            score_group(pt_i, hq, sb, g)
                        for j in range(4):
                            consume_t(pt_p, rps, ups, 4 * g + j)
                    normalize(rps, ups, phq, psb)
                elif cur is not None:
                    for g in range(4):
                        score_group(pt_i, hq, sb, g)
                prev = cur

        # ---- Phase C: partial O-projection ----
        with ExitStack() as cc:
            op = cc.enter_context(tc.tile_pool(name="op", bufs=4))
            pc = cc.enter_context(tc.tile_pool(name="pc", bufs=1, space="PSUM"))

            for st in range(NTT):
                stl = slice(st * P, (st + 1) * P)
                pss = [pc.tile([P, 512], f32, tag=f"pc{nb}", name=f"pc{nb}") for nb in range(8)]
                for kc in range(QH):
                    for nb in range(8):
                        nc.tensor.matmul(
                            pss[nb][:],
                            lhsT=at[kc][:, stl],
                            rhs=wo_sb[:, kc, nb * 512 : (nb + 1) * 512],
                            start=(kc == 0),
                            stop=(kc == QH - 1),
                        )
                for nb in range(8):
                    ot = op.tile([P, 512], bf, tag="ot")
                    nc.scalar.activation(ot[:], pss[nb][:], AF.Copy)
                    nc.sync.dma_start(
                        out_d[:][stl, nb * 512 : (nb + 1) * 512], ot[:]
                    )

    nc.compile()
    return nc


def _get_nc():
    if "nc" not in _CACHE:
        _CACHE["nc"] = _build()
    return _CACHE["nc"]


def _run(in_maps, trace=False, **kw):
    from concourse import bass_utils

    nc = _get_nc()
    return bass_utils.run_bass_kernel_spmd(
        nc, in_maps, core_ids=list(range(NCORES)), trace=trace, **kw
    )


def make_in_maps(hidden_states, Wq, Wk, Wv, Wo, bq, bk):
    x = np.asarray(hidden_states, np.float32).reshape(S, H)
    xt = np.ascontiguousarray(x.T).astype(BF16)
    in_maps = []
    for c in range(NCORES):
        bias = np.zeros((P, 6), np.float32)
        for i in range(QH):
            bias[:, i] = np.asarray(bq, np.float32)[c * DQ + i * HD : c * DQ + (i + 1) * HD]
        bias[:, 4] = np.asarray(bk, np.float32)[c * HD : (c + 1) * HD]
        in_maps.append(
            {
                "xt": xt,
                "wq": np.ascontiguousarray(np.asarray(Wq, np.float32)[:, c * DQ : (c + 1) * DQ]).astype(BF16),
                "wk": np.ascontiguousarray(np.asarray(Wk, np.float32)[:, c * HD : (c + 1) * HD]).astype(BF16),
                "wv": np.ascontiguousarray(np.asarray(Wv, np.float32)[:, c * HD : (c + 1) * HD]).astype(BF16),
                "wo": np.ascontiguousarray(np.asarray(Wo, np.float32)[c * DQ : (c + 1) * DQ, :]).astype(BF16),
                "bias": bias,
            }
        )
    return in_maps


def _gather(results, Wo, bv, bo):
    out = np.zeros((S, H), np.float32)
    for c in range(NCORES):
        out += results[c]["out"].astype(np.float32)
    bv = np.asarray(bv, np.float32)
    rep_bv = np.concatenate(
        [np.tile(bv[c * HD : (c + 1) * HD], QH) for c in range(NCORES)]
    )
    out += rep_bv @ np.asarray(Wo, np.float32) + np.asarray(bo, np.float32)
    return out.reshape(1, S, H)


def _numpy_fallback(hidden_states, attention_mask, Wq, bq, Wk, bk, Wv, bv, Wo, bo):
    x = np.asarray(hidden_states, np.float32)
    b, s, _ = x.shape
    n_rep = NH // NKV
    q = (x @ Wq + bq).reshape(b, s, NH, HD)
    k = (x @ Wk + bk).reshape(b, s, NKV, HD)
    v = (x @ Wv + bv).reshape(b, s, NKV, HD)
    k = np.repeat(k, n_rep, axis=2).transpose(0, 2, 1, 3)
    v = np.repeat(v, n_rep, axis=2).transpose(0, 2, 1, 3)
    q = q.transpose(0, 2, 1, 3)
    sc = np.einsum("bhsd,bhtd->bhst", q, k) * SCALE + np.asarray(attention_mask, np.float32)
    sc -= sc.max(-1, keepdims=True)
    p = np.exp(sc)
    p /= p.sum(-1, keepdims=True)
    o = np.einsum("bhst,bhtd->bhsd", p, v).transpose(0, 2, 1, 3).reshape(b, s, NH * HD)
    return (o @ Wo + bo).astype(np.float32)


def kernel(hidden_states, attention_mask, Wq, bq, Wk, bk, Wv, bv, Wo, bo):
    hidden_states = np.asarray(hidden_states)
    if (
        hidden_states.shape != (1, S, H)
        or np.any(np.asarray(attention_mask))
    ):
        return _numpy_fallback(
            hidden_states, attention_mask, Wq, bq, Wk, bk, Wv, bv, Wo, bo
        )
    in_maps = make_in_maps(hidden_states, Wq, Wk, Wv, Wo, bq, bk)
    res = _run(in_maps)
    return _gather(res.results, Wo, bv, bo).astype(np.float32)


# revision 9
# speedup vs baseline: 1.0441x; 1.0441x over previous
"""Tensor-parallel GQA attention layer for 8 TRN2 NeuronCores.

Sharding: TP over heads. Core c owns q-heads 4c..4c+3 (512 cols of Wq),
kv-head c (128 cols of Wk/Wv), and rows 512c..512c+512 of Wo. Activations
(hidden_states) are replicated; each core emits a partial O-projection
[2048, 4096] which the host sums.

On-device math (per core), all matmuls bf16 with fp32 PSUM accumulate:
  QT[d,s] = Wq_c^T X^T   (4 tiles of [128,2048])
  KT[d,s] = Wk_c^T X^T ; VT[d,s] = Wv_c^T X^T ; V = VT^T via PE transpose
  per (head, s-block of 512):
    S^T[t,s]  = K Q_h^T           (16 t-chunk matmuls, K^T chunks stationary)
    P^T       = exp(S^T * scale)  (ACT, psum->sbuf bf16)
    r[1,s]    = ones^T P^T        (ones stationary, accumulate over t)
    U^T[d,s]  = V^T P^T           (V chunks stationary, accumulate over t)
    A^T       = U^T * bcast(1/r)  (PE K=1 broadcast matmul + DVE multiply)
  out[s,:]   += A^T_chunks^T Wo_c  (partial; host sums cores, adds bo/bv terms)
"""

import sys

if "/opt/trn_rl_repo" not in sys.path:
    sys.path.insert(0, "/opt/trn_rl_repo")

from contextlib import ExitStack

import ml_dtypes
import numpy as np

H = 4096
NH = 32
NKV = 8
HD = 128
S = 2048
NCORES = 8
QH = NH // NCORES          # 4 q heads per core
DQ = QH * HD               # 512
SCALE = float(HD) ** -0.5
P = 128
NSB = S // 512             # 4 s-blocks
NHC = H // P               # 32 hidden chunks
NTT = S // P               # 16 t-chunks
BF16 = ml_dtypes.bfloat16

_CACHE: dict = {}


def _build():
    import concourse.bacc as bacc
    import concourse.mybir as mybir
    import concourse.tile as tile
    from concourse.masks import make_identity

    bf = mybir.dt.bfloat16
    f32 = mybir.dt.float32
    AF = mybir.ActivationFunctionType
    ALU = mybir.AluOpType

    nc = bacc.Bacc("TRN2", target_bir_lowering=False, debug=False)

    xt_d = nc.dram_tensor("xt", [H, S], bf, kind="ExternalInput")
    wq_d = nc.dram_tensor("wq", [H, DQ], bf, kind="ExternalInput")
    wk_d = nc.dram_tensor("wk", [H, HD], bf, kind="ExternalInput")
    wv_d = nc.dram_tensor("wv", [H, HD], bf, kind="ExternalInput")
    wo_d = nc.dram_tensor("wo", [DQ, H], bf, kind="ExternalInput")
    bias_d = nc.dram_tensor("bias", [P, 6], f32, kind="ExternalInput")
    out_d = nc.dram_tensor("out", [S, H], bf, kind="ExternalOutput")

    with tile.TileContext(nc) as tc, ExitStack() as ctx:
        const = ctx.enter_context(tc.tile_pool(name="const", bufs=1))
        ones_col = const.tile([P, 1], bf)
        nc.vector.memset(ones_col[:], 1.0)
        ones_row = const.tile([1, P], bf)
        nc.vector.memset(ones_row[:], 1.0)
        ident = const.tile([P, P], bf)
        make_identity(nc, ident[:])
        zbias = const.tile([P, 1], f32)
        nc.vector.memset(zbias[:], 0.0)
        bias_sb = const.tile([P, 6], f32)
        nc.sync.dma_start(bias_sb[:], bias_d[:])

        keep = ctx.enter_context(tc.tile_pool(name="keep", bufs=1))
        qt = [keep.tile([P, S], bf, tag=f"qt{i}", name=f"qt{i}") for i in range(QH)]
        kt = keep.tile([P, S], bf, tag="kt")
        vt = keep.tile([P, S], bf, tag="vt")
        vv = keep.tile([P, NTT, P], bf, tag="vv")      # V chunks [t,d]
        at = [keep.tile([P, S], bf, tag=f"at{i}", name=f"at{i}") for i in range(QH)]

        # ---- Phase A: projections ----
        with ExitStack() as ca:
            xp = ca.enter_context(tc.tile_pool(name="xp", bufs=2))
            wp = ca.enter_context(tc.tile_pool(name="wp", bufs=1))
            pa = ca.enter_context(tc.tile_pool(name="pa", bufs=4, space="PSUM"))
            tp = ca.enter_context(tc.tile_pool(name="tp", bufs=2, space="PSUM"))

            wq_sb = wp.tile([P, NHC, DQ], bf)
            wk_sb = wp.tile([P, NHC, HD], bf)
            wv_sb = wp.tile([P, NHC, HD], bf)
            wq_v = wq_d[:].rearrange("(a p) n -> p a n", p=P)
            for g in range(4):
                gs = slice(g * 8, (g + 1) * 8)
                nc.sync.dma_start(wq_sb[:, gs, :], wq_v[:, gs, :])
            nc.sync.dma_start(wk_sb[:], wk_d[:].rearrange("(a p) n -> p a n", p=P))
            nc.sync.dma_start(wv_sb[:], wv_d[:].rearrange("(a p) n -> p a n", p=P))

            xt_v = xt_d[:].rearrange("(a p) s -> p a s", p=P)
            for sb in range(NSB):
                sl = slice(sb * 512, (sb + 1) * 512)
                xs = xp.tile([P, NHC, 512], bf, tag="xs")
                for g in range(4):
                    gs = slice(g * 8, (g + 1) * 8)
                    nc.sync.dma_start(xs[:, gs, :], xt_v[:, gs, sl])
                projs = (
                    [(wq_sb, i * HD, qt[i], i) for i in range(QH)]
                    + [(wk_sb, 0, kt, 4), (wv_sb, 0, vt, 5)]
                )
                for w_sb, off, dst, bcol in projs:
                    ps = pa.tile([P, 512], f32, tag="pa")
                    for h in range(NHC):
                        nc.tensor.matmul(
                            ps[:],
                            lhsT=w_sb[:, h, off : off + HD],
                            rhs=xs[:, h, :],
                            start=(h == 0),
                            stop=(h == NHC - 1),
                        )
                    nc.scalar.activation(
                        dst[:, sl], ps[:], AF.Identity, bias=bias_sb[:, bcol : bcol + 1]
                    )

            # V = VT^T via PE transpose (16 chunks)
            for t in range(NTT):
                pt_ps = tp.tile([P, P], bf, tag="tp")
                nc.tensor.transpose(pt_ps[:], vt[:, t * P : (t + 1) * P], ident[:])
                nc.vector.tensor_copy(vv[:, t, :], pt_ps[:])

        # Wo staged early so its DMA overlaps phase B compute.
        wop = ctx.enter_context(tc.tile_pool(name="wop", bufs=1))
        wo_sb = wop.tile([P, QH, H], bf)
        nc.sync.dma_start(wo_sb[:], wo_d[:].rearrange("(a p) n -> p a n", p=P))

        # ---- Phase B: attention, software-pipelined over 16 (head, s-block)
        # blocks. Score matmuls of block i interleave with rowsum/AV matmuls
        # of block i-1 so the PE never waits on ACT's exp. Exp runs on
        # 4-chunk groups ([128, 2048]) to amortize ACT fixed overhead. ----
        with ExitStack() as cb:
            pb = cb.enter_context(tc.tile_pool(name="pb", bufs=1))
            ss = cb.enter_context(tc.tile_pool(name="ss", bufs=1, space="PSUM"))
            us = cb.enter_context(tc.tile_pool(name="us", bufs=2, space="PSUM"))
            rs = cb.enter_context(tc.tile_pool(name="rs", bufs=1, space="PSUM"))
            bs = cb.enter_context(tc.tile_pool(name="bs", bufs=1, space="PSUM"))

            blocks = [(hq, sb) for hq in range(QH) for sb in range(NSB)]

            def score_group(pt_i, hq, sb, g):
                sl = slice(sb * 512, (sb + 1) * 512)
                sps = ss.tile([P, 4, 512], f32, tag="ss", name="sps")
                for j in range(4):
                    t = 4 * g + j
                    nc.tensor.matmul(
                        sps[:, j, :],
                        lhsT=kt[:, t * P : (t + 1) * P],
                        rhs=qt[hq][:, sl],
                        start=True,
                        stop=True,
                    )
                nc.scalar.activation(
                    pt_i[:, 4 * g : 4 * g + 4, :],
                    sps[:],
                    AF.Exp,
                    bias=zbias[:],
                    scale=SCALE,
                )

            def consume_t(pt_p, rps, ups, t):
                nc.tensor.matmul(
                    rps[:],
                    lhsT=ones_col[:],
                    rhs=pt_p[:, t, :],
                    start=(t == 0),
                    stop=(t == NTT - 1),
                )
                nc.tensor.matmul(
                    ups[:],
                    lhsT=vv[:, t, :],
                    rhs=pt_p[:, t, :],
                    start=(t == 0),
                    stop=(t == NTT - 1),
                )

            def normalize(rps, ups, hq, sb):
                sl = slice(sb * 512, (sb + 1) * 512)
                recip = pb.tile([1, 512], f32, tag="recip", bufs=2, name="recip")
                nc.vector.reciprocal(recip[:], rps[:])
                recip_bf = pb.tile([1, 512], bf, tag="recipb", bufs=2, name="recipb")
                nc.vector.tensor_copy(recip_bf[:], recip[:])
                bps = bs.tile([P, 512], f32, tag="bs", name="bps")
                nc.tensor.matmul(
                    bps[:], lhsT=ones_row[:], rhs=recip_bf[:], start=True, stop=True
                )
                bcast = pb.tile([P, 512], f32, tag="bcast", bufs=2, name="bcast")
                nc.vector.tensor_copy(bcast[:], bps[:])
                nc.vector.scalar_tensor_tensor(
                    out=at[hq][:, sl],
                    in0=ups[:],
                    scalar=1.0,
                    in1=bcast[:],
                    op0=ALU.mult,
                    op1=ALU.mult,
                )

            prev = None
            for i in range(len(blocks) + 1):
                cur = None
                if i < len(blocks):
                    hq, sb = blocks[i]
                    pt_i = pb.tile([P, NTT, 512], bf, tag="pt", bufs=2, name="pt")
                    cur = (pt_i, hq, sb)
                if prev is not None:
                    pt_p, phq, psb = prev
                    rps = rs.tile([1, 512], f32, tag="rs", name="rps")
                    ups = us.tile([P, 512], f32, tag="us", name="ups")
                    for g in range(4):
                        if cur is not None:
                            score_group(pt_i, hq, sb, g)
                        for j in range(4):
                            consume_t(pt_p, rps, ups, 4 * g + j)
                    normalize(rps, ups, phq, psb)
                elif cur is not None:
                    for g in range(4):
                        score_group(pt_i, hq, sb, g)
                prev = cur

        # ---- Phase C: partial O-projection ----
        with ExitStack() as cc:
            op = cc.enter_context(tc.tile_pool(name="op", bufs=4))
            pc = cc.enter_context(tc.tile_pool(name="pc", bufs=1, space="PSUM"))

            for st in range(NTT):
                stl = slice(st * P, (st + 1) * P)
                pss = [pc.tile([P, 512], f32, tag=f"pc{nb}", name=f"pc{nb}") for nb in range(8)]
                for kc in range(QH):
                    for nb in range(8):
                        nc.tensor.matmul(
                            pss[nb][:],
                            lhsT=at[kc][:, stl],
                            rhs=wo_sb[:, kc, nb * 512 : (nb + 1) * 512],
                            start=(kc == 0),
                            stop=(kc == QH - 1),
                        )
                for nb in range(8):
                    ot = op.tile([P, 512], bf, tag="ot")
                    nc.scalar.activation(ot[:], pss[nb][:], AF.Copy)
                    nc.sync.dma_start(
                        out_d[:][stl, nb * 512 : (nb + 1) * 512], ot[:]
                    )

    nc.compile()
    return nc


def _get_nc():
    if "nc" not in _CACHE:
        _CACHE["nc"] = _build()
    return _CACHE["nc"]


def _run(in_maps, trace=False, **kw):
    from concourse import bass_utils

    nc = _get_nc()
    return bass_utils.run_bass_kernel_spmd(
        nc, in_maps, core_ids=list(range(NCORES)), trace=trace, **kw
    )


def make_in_maps(hidden_states, Wq, Wk, Wv, Wo, bq, bk):
    x = np.asarray(hidden_states, np.float32).reshape(S, H)
    xt = np.ascontiguousarray(x.T).astype(BF16)
    in_maps = []
    for c in range(NCORES):
        bias = np.zeros((P, 6), np.float32)
        for i in range(QH):
            bias[:, i] = np.asarray(bq, np.float32)[c * DQ + i * HD : c * DQ + (i + 1) * HD]
        bias[:, 4] = np.asarray(bk, np.float32)[c * HD : (c + 1) * HD]
        in_maps.append(
            {
                "xt": xt,
                "wq": np.ascontiguousarray(np.asarray(Wq, np.float32)[:, c * DQ : (c + 1) * DQ]).astype(BF16),
                "wk": np.ascontiguousarray(np.asarray(Wk, np.float32)[:, c * HD : (c + 1) * HD]).astype(BF16),
                "wv": np.ascontiguousarray(np.asarray(Wv, np.float32)[:, c * HD : (c + 1) * HD]).astype(BF16),
                "wo": np.ascontiguousarray(np.asarray(Wo, np.float32)[c * DQ : (c + 1) * DQ, :]).astype(BF16),
                "bias": bias,
            }
        )
    return in_maps


def _gather(results, Wo, bv, bo):
    out = np.zeros((S, H), np.float32)
    for c in range(NCORES):
        out += results[c]["out"].astype(np.float32)
    bv = np.asarray(bv, np.float32)
    rep_bv = np.concatenate(
        [np.tile(bv[c * HD : (c + 1) * HD], QH) for c in range(NCORES)]
    )
    out += rep_bv @ np.asarray(Wo, np.float32) + np.asarray(bo, np.float32)
    return out.reshape(1, S, H)


def _numpy_fallback(hidden_states, attention_mask, Wq, bq, Wk, bk, Wv, bv, Wo, bo):
    x = np.asarray(hidden_states, np.float32)
    b, s, _ = x.shape
    n_rep = NH // NKV
    q = (x @ Wq + bq).reshape(b, s, NH, HD)
    k = (x @ Wk + bk).reshape(b, s, NKV, HD)
    v = (x @ Wv + bv).reshape(b, s, NKV, HD)
    k = np.repeat(k, n_rep, axis=2).transpose(0, 2, 1, 3)
    v = np.repeat(v, n_rep, axis=2).transpose(0, 2, 1, 3)
    q = q.transpose(0, 2, 1, 3)
    sc = np.einsum("bhsd,bhtd->bhst", q, k) * SCALE + np.asarray(attention_mask, np.float32)
    sc -= sc.max(-1, keepdims=True)
    p = np.exp(sc)
    p /= p.sum(-1, keepdims=True)
    o = np.einsum("bhst,bhtd->bhsd", p, v).transpose(0, 2, 1, 3).reshape(b, s, NH * HD)
    return (o @ Wo + bo).astype(np.float32)


def kernel(hidden_states, attention_mask, Wq, bq, Wk, bk, Wv, bv, Wo, bo):
    hidden_states = np.asarray(hidden_states)
    if (
        hidden_states.shape != (1, S, H)
        or np.any(np.asarray(attention_mask))
    ):
        return _numpy_fallback(
            hidden_states, attention_mask, Wq, bq, Wk, bk, Wv, bv, Wo, bo
        )
    in_maps = make_in_maps(hidden_states, Wq, Wk, Wv, Wo, bq, bk)
    res = _run(in_maps)
    return _gather(res.results, Wo, bv, bo).astype(np.float32)
